# revision 41
# baseline (speedup 1.0000x reference)
"""MultiHeadAttention Trainium2 Bass kernel, 8-core SPMD.

Problem: B=4, S=2048, EMBED=1024, HEADS=16, HEAD_DIM=64 (fp32).
Sharding: core c -> batch b=c//2, query-half h=c%2 (1024 query rows).
Each core computes its 1024 output rows end-to-end; no collectives.

Per-core dataflow (all layouts transposed: feature/kk dim on partitions):
  A1: KT = WkT.T @ XkT + bk              -> SBUF resident  (f32r)
  A2: V  = XvT.T @ WvT (natural [kk,d])  -> SBUF resident bf16, with a
      ones column per head (V_aug) so the PV matmul also produces the
      softmax denominator for free.
  B:  per head-pair p:
      Q-proj for the pair (QT tile stays in SBUF; scale 1/8 + bias
      folded into the PSUM evacuation), then per q-chunk qc(512) and
      kk-pair kkp:
        S.T[kk,q] = KT_h.T @ QT_h        (f32r, heads row-packed in PE)
        P = exp(S.T)                      (ACT, no max-subtraction --
                                           scores ~ N(0,1), safe)
        P *= (1-mask).T                   (DVE, bf16 2x mode)
        OT_h[65,512] += V_aug_h.T @ P     (bf16; row 64 = denominator)
      normalize: OT_h[0:64] * recip(OT_h[64]) -> ot_dram (f32r)
  C:  outT = WoT.T @ OT + (bo + Wo @ bv)  -> DRAM [1024,1024]
Host reassembles out[b, h*1024:(h+1)*1024, :] = outT.T per core.
"""
import numpy as np
import ml_dtypes

import concourse.bass as bass
import concourse.mybir as mybir
import concourse.tile as tile
from concourse import bacc
from concourse.bass_utils import run_bass_kernel_spmd

F32R = mybir.dt.float32r
F32 = mybir.dt.float32
BF16 = mybir.dt.bfloat16
Act = mybir.ActivationFunctionType
Alu = mybir.AluOpType

EMBED = 1024
HEADS = 16
DH = 64
SQ = 1024   # query rows per core
SK = 2048   # key rows per core
NF = 8      # feature tiles (1024/128)
NKT = 16    # kk tiles (2048/128)
N_CORES = 8

_STATE = {}


def build_nc():
    nc = bacc.Bacc("TRN2", target_bir_lowering=False)
    xqT = nc.dram_tensor("xqT", [EMBED, SQ], F32R, kind="ExternalInput")
    xkT = nc.dram_tensor("xkT", [EMBED, SK], F32R, kind="ExternalInput")
    xvT = nc.dram_tensor("xvT", [EMBED, SK], F32R, kind="ExternalInput")
    wqT = nc.dram_tensor("wqT", [EMBED, EMBED], F32R, kind="ExternalInput")
    wkT = nc.dram_tensor("wkT", [EMBED, EMBED], F32R, kind="ExternalInput")
    wvT = nc.dram_tensor("wvT", [EMBED, EMBED], F32R, kind="ExternalInput")
    woT = nc.dram_tensor("woT", [EMBED, EMBED], F32R, kind="ExternalInput")
    bq8 = nc.dram_tensor("bq8", [128, NF], F32, kind="ExternalInput")
    bk_l = nc.dram_tensor("bk_l", [128, NF], F32, kind="ExternalInput")
    bo2 = nc.dram_tensor("bo2", [128, NF], F32, kind="ExternalInput")
    notmT = nc.dram_tensor("notmT", [SK, SQ], BF16, kind="ExternalInput")
    outT = nc.dram_tensor("outT", [EMBED, SQ], F32, kind="ExternalOutput")
    ot_dram = nc.dram_tensor("ot_dram", [EMBED, SQ], F32R)

    xqT_r = xqT.rearrange("(t p) q -> p t q", p=128)
    xkT_r = xkT.rearrange("(t p) k -> p t k", p=128)
    xvT_r = xvT.rearrange("(t p) k -> p t k", p=128)
    wqT_r = wqT.rearrange("(t p) n -> p t n", p=128)
    wkT_r = wkT.rearrange("(t p) n -> p t n", p=128)
    wvT_r = wvT.rearrange("(t p) n -> p t n", p=128)
    woT_r = woT.rearrange("(t p) n -> p t n", p=128)
    notmT_r = notmT.rearrange("(t p) q -> p t q", p=128)
    ot_dram_r = ot_dram.rearrange("(t p) q -> p t q", p=128)

    with tile.TileContext(nc) as tc:
        # ---------- persistent + early-prefetch pools ----------
        with tc.tile_pool(name="persist", bufs=1) as pp, \
             tc.tile_pool(name="bias", bufs=1) as bp, \
             tc.tile_pool(name="xv", bufs=2) as xvpool, \
             tc.tile_pool(name="bwq", bufs=2) as wqpool:
            kt = pp.tile([128, NF, SK], F32R, name="kt")
            vaug = pp.tile([128, NKT, HEADS * 65], BF16, name="vaug")
            bq8_sb = bp.tile([128, NF], F32, name="bq8_sb")
            bk_sb = bp.tile([128, NF], F32, name="bk_sb")
            bo2_sb = bp.tile([128, NF], F32, name="bo2_sb")

            # ---------- phase A1: K projection -> kt ----------
            with tc.tile_pool(name="a1w", bufs=1) as wpool, \
                 tc.tile_pool(name="a1x", bufs=2) as xpool, \
                 tc.tile_pool(name="a1p", bufs=4, space="PSUM") as pspool:
                wk_sb = wpool.tile([128, NF, EMBED], F32R, name="wk_sb")
                for ck in range(4):
                    xk_sb = xpool.tile([128, NF, 512], F32R, name="xk_sb")
                    if ck == 0:
                        nc.sync.dma_start(out=xk_sb[:, 0:4, :],
                                          in_=xkT_r[:, 0:4, 0:512])
                        nc.sync.dma_start(out=wk_sb[:, :, 0:256],
                                          in_=wkT_r[:, :, 0:256])
                        nc.sync.dma_start(out=xk_sb[:, 4:8, :],
                                          in_=xkT_r[:, 4:8, 0:512])
                        nc.sync.dma_start(out=bk_sb[:], in_=bk_l[:, :])
                        nc.sync.dma_start(out=bq8_sb[:], in_=bq8[:, :])
                        nc.sync.dma_start(out=bo2_sb[:], in_=bo2[:, :])
                        for c4 in range(1, 4):
                            nc.sync.dma_start(
                                out=wk_sb[:, :, c4 * 256:(c4 + 1) * 256],
                                in_=wkT_r[:, :, c4 * 256:(c4 + 1) * 256])
                    else:
                        nc.sync.dma_start(
                            out=xk_sb[:],
                            in_=xkT_r[:, :, ck * 512:(ck + 1) * 512])
                    for m in range(NF):
                        ps = pspool.tile([128, 512], F32, name="a1ps")
                        for fi in range(NF):
                            nc.tensor.matmul(
                                ps[:], wk_sb[:, fi, m * 128:(m + 1) * 128],
                                xk_sb[:, fi, :],
                                start=(fi == 0), stop=(fi == NF - 1))
                        nc.vector.tensor_scalar(
                            out=kt[:, m, ck * 512:(ck + 1) * 512],
                            in0=ps[:], scalar1=bk_sb[:, m:m + 1],
                            scalar2=None, op0=Alu.add)

            # ---------- phase A2: V projection -> vaug (bf16 + ones) ----------
            # n-outer: heads 0..7 (n=0) complete first so phase B's first
            # head-pairs can overlap with the n=1 half.
            vaug_r = vaug.rearrange("p k (h c) -> p k h c", c=65)
            nc.vector.memset(vaug_r[:, :, :, 64:65], 1.0)
            xq_ctx = tc.tile_pool(name="xq", bufs=1)
            xqpool = xq_ctx.__enter__()
            xq_sb = xqpool.tile([128, NF, SQ], F32R, name="xq_sb")
            nc.sync.dma_start(out=xq_sb[:, :, 0:512], in_=xqT_r[:, :, 0:512])
            nc.sync.dma_start(out=xq_sb[:, :, 512:1024],
                              in_=xqT_r[:, :, 512:1024])
            with tc.tile_pool(name="a2w", bufs=2) as wpool, \
                 tc.tile_pool(name="a2p", bufs=4, space="PSUM") as pspool:
                wv_sb = []
                for n in range(2):
                    t = wpool.tile([128, NF, 512], F32R, name="wv_sb")
                    nc.sync.dma_start(out=t[:],
                                      in_=wvT_r[:, :, n * 512:(n + 1) * 512])
                    wv_sb.append(t)
                for m in range(NKT):
                    xv_sb = xvpool.tile([128, NF, 128], F32R,
                                        name="xv_sb")
                    nc.sync.dma_start(
                        out=xv_sb[:],
                        in_=xvT_r[:, :, m * 128:(m + 1) * 128])
                    for n in range(2):
                        ps = pspool.tile([128, 512], F32, name="a2ps")
                        for fi in range(NF):
                            nc.tensor.matmul(
                                ps[:], xv_sb[:, fi, :],
                                wv_sb[n][:, fi, :],
                                start=(fi == 0), stop=(fi == NF - 1))
                        nc.vector.tensor_copy(
                            out=vaug_r[:, m, n * 8:(n + 1) * 8, 0:64],
                            in_=ps.rearrange("p (h c) -> p h c", c=64))

            # ---------- phase B: Q-proj + attention, per head pair ----------
            with tc.tile_pool(name="bnotm", bufs=1) as nmpool, \
                 tc.tile_pool(name="bqt", bufs=2) as qpool, \
                 tc.tile_pool(name="bpt", bufs=2) as ptpool, \
                 tc.tile_pool(name="bnrm", bufs=2) as npool, \
                 tc.tile_pool(name="bst", bufs=1, space="PSUM") as stpool, \
                 tc.tile_pool(name="bqp", bufs=2, space="PSUM") as qppool, \
                 tc.tile_pool(name="bot", bufs=1, space="PSUM") as otpool:
                wq_first = wqpool.tile([128, NF, 128], F32R, name="wq_sb",
                                       tag="wq_sb")
                nc.sync.dma_start(out=wq_first[:], in_=wqT_r[:, :, 0:128])
                notm = nmpool.tile([128, NKT, SQ], BF16, name="notm")
                for c4 in range(4):
                    nc.sync.dma_start(
                        out=notm[:, c4 * 4:(c4 + 1) * 4, :],
                        in_=notmT_r[:, c4 * 4:(c4 + 1) * 4, :])
                for p in range(8):  # head pairs
                    # Q projection for this pair -> qt_sb [128, 1024] f32r
                    if p == 0:
                        wq_sb = wq_first
                    else:
                        wq_sb = wqpool.tile([128, NF, 128], F32R,
                                            name="wq_sb", tag="wq_sb")
                        nc.sync.dma_start(
                            out=wq_sb[:],
                            in_=wqT_r[:, :, p * 128:(p + 1) * 128])
                    qt_sb = qpool.tile([128, SQ], F32R, name="qt_sb")
                    for qc in range(2):
                        qps = qppool.tile([128, 512], F32, name="qps")
                        for fi in range(NF):
                            nc.tensor.matmul(
                                qps[:], wq_sb[:, fi, :],
                                xq_sb[:, fi, qc * 512:(qc + 1) * 512],
                                start=(fi == 0), stop=(fi == NF - 1))
                        nc.vector.tensor_scalar(
                            out=qt_sb[:, qc * 512:(qc + 1) * 512], in0=qps[:],
                            scalar1=0.125, scalar2=bq8_sb[:, p:p + 1],
                            op0=Alu.mult, op1=Alu.add)
                    for qc in range(2):
                        otps = [otpool.tile([128, 512], F32,
                                            name=f"otps{j}", tag=f"otps{j}")
                                for j in range(2)]
                        for kkp in range(8):
                            sts = [stpool.tile([128, 1024], F32,
                                               name=f"stps{j}", tag=f"stps{j}")
                                   for j in range(2)]
                            # ST matmuls interleaved by head so adjacent
                            # PE ops target disjoint row groups (0,0)/(64,0)
                            # and run concurrently (MMs are strict FIFO --
                            # same-row-group neighbors serialize).
                            def st_mm(hh, j):
                                lo = hh * 64
                                kkt = 2 * kkp + j
                                nc.tensor.matmul(
                                    sts[hh][:, j * 512:(j + 1) * 512],
                                    kt[lo:lo + 64, p,
                                       kkt * 128:(kkt + 1) * 128],
                                    qt_sb[lo:lo + 64,
                                          qc * 512:(qc + 1) * 512],
                                    start=True, stop=True,
                                    tile_position=(lo, 0))
                            pts = []
                            st_mm(0, 0)
                            st_mm(1, 0)
                            st_mm(0, 1)
                            pt0 = ptpool.tile([128, 1024], BF16,
                                              name="pt0", tag="pt0")
                            nc.scalar.activation(pt0[:], sts[0][:], Act.Exp)
                            pts.append(pt0)
                            st_mm(1, 1)
                            pt1 = ptpool.tile([128, 1024], BF16,
                                              name="pt1", tag="pt1")
                            nc.scalar.activation(pt1[:], sts[1][:], Act.Exp)
                            pts.append(pt1)
                            for hh in range(2):
                                h = 2 * p + hh
                                for j in range(2):
                                    kkt = 2 * kkp + j
                                    nc.vector.tensor_tensor(
                                        out=pts[hh][:, j * 512:(j + 1) * 512],
                                        in0=pts[hh][:, j * 512:(j + 1) * 512],
                                        in1=notm[:, kkt,
                                                 qc * 512:(qc + 1) * 512],
                                        op=Alu.mult)
                                    nc.tensor.matmul(
                                        otps[hh][0:65, :],
                                        vaug_r[:, kkt, h, :],
                                        pts[hh][:, j * 512:(j + 1) * 512],
                                        start=(kkp == 0 and j == 0),
                                        stop=(kkp == 7 and j == 1))
                        for hh in range(2):
                            rec = npool.tile([1, 512], F32, name="rec",
                                             tag="rec")
                            nc.vector.reciprocal(rec[:], otps[hh][64:65, :])
                            recb = npool.tile([64, 512], F32, name="recb",
                                              tag="recb")
                            nc.gpsimd.partition_broadcast(recb[:], rec[:])
                            otstg = npool.tile([64, 512], F32R, name="otstg",
                                               tag="otstg")
                            nc.vector.tensor_tensor(
                                out=otstg[:],
                                in0=otps[hh][0:64, :], in1=recb[:],
                                op=Alu.mult)
                            nc.sync.dma_start(
                                out=ot_dram[p * 128 + hh * 64:
                                            p * 128 + hh * 64 + 64,
                                            qc * 512:(qc + 1) * 512],
                                in_=otstg[:])

            # ---------- phase C: output projection ----------
            # wo streams through the (still open) bwq pool so the first
            # blocks prefetch during phase B's tail.
            with tc.tile_pool(name="cot", bufs=1) as cotpool, \
                 tc.tile_pool(name="cs", bufs=3) as spool, \
                 tc.tile_pool(name="cp", bufs=4, space="PSUM") as pspool:
                ot_sb = []
                for qc in range(2):
                    t = cotpool.tile([128, NF, 512], F32R, name=f"ot_sb{qc}")
                    if qc == 0:
                        nc.sync.dma_start(out=t[:, 0:4, :],
                                          in_=ot_dram_r[:, 0:4, 0:512])
                        nc.sync.dma_start(out=t[:, 4:8, :],
                                          in_=ot_dram_r[:, 4:8, 0:512])
                    else:
                        nc.sync.dma_start(
                            out=t[:],
                            in_=ot_dram_r[:, :, qc * 512:(qc + 1) * 512])
                    ot_sb.append(t)
                for m in range(NF):
                    wo_sb = wqpool.tile([128, NF, 128], F32R, name="wo_sb",
                                        tag="wq_sb")
                    nc.sync.dma_start(
                        out=wo_sb[:],
                        in_=woT_r[:, :, m * 128:(m + 1) * 128])
                    for qc in range(2):
                        ps = pspool.tile([128, 512], F32, name="cps")
                        for fi in range(NF):
                            nc.tensor.matmul(
                                ps[:], wo_sb[:, fi, :],
                                ot_sb[qc][:, fi, :],
                                start=(fi == 0), stop=(fi == NF - 1))
                        stg = spool.tile([128, 512], F32, name="cstg")
                        nc.vector.tensor_scalar(
                            out=stg[:], in0=ps[:],
                            scalar1=bo2_sb[:, m:m + 1], scalar2=None,
                            op0=Alu.add)
                        nc.sync.dma_start(
                            out=outT[m * 128:(m + 1) * 128,
                                     qc * 512:(qc + 1) * 512],
                            in_=stg[:])
            xq_ctx.__exit__(None, None, None)
    nc.compile()
    return nc


def _get_nc():
    if "nc" not in _STATE:
        _STATE["nc"] = build_nc()
    return _STATE["nc"]


def kernel(query, key, value, mask, Wq, bq, Wk, bk, Wv, bv, Wo, bo):
    query = np.asarray(query, dtype=np.float32)
    key = np.asarray(key, dtype=np.float32)
    value = np.asarray(value, dtype=np.float32)
    mask = np.asarray(mask)
    Wq = np.asarray(Wq, dtype=np.float32)
    Wk = np.asarray(Wk, dtype=np.float32)
    Wv = np.asarray(Wv, dtype=np.float32)
    Wo = np.asarray(Wo, dtype=np.float32)
    bq = np.asarray(bq, dtype=np.float32)
    bk = np.asarray(bk, dtype=np.float32)
    bv = np.asarray(bv, dtype=np.float32)
    bo = np.asarray(bo, dtype=np.float32)

    wqT = np.ascontiguousarray(Wq.T)
    wkT = np.ascontiguousarray(Wk.T)
    wvT = np.ascontiguousarray(Wv.T)
    woT = np.ascontiguousarray(Wo.T)
    bq8 = np.ascontiguousarray((bq / 8.0).reshape(NF, 128).T)
    bk_l = np.ascontiguousarray(bk.reshape(NF, 128).T)
    bo2v = bo + Wo @ bv
    bo2 = np.ascontiguousarray(bo2v.reshape(NF, 128).T)

    in_maps = []
    for c in range(N_CORES):
        b, h = c // 2, c % 2
        rows = slice(h * SQ, (h + 1) * SQ)
        xqTc = np.ascontiguousarray(query[b, rows, :].T)
        xkTc = np.ascontiguousarray(key[b].T)
        xvTc = np.ascontiguousarray(value[b].T)
        notm = np.ascontiguousarray(
            (~mask[b, 0, rows, :]).T.astype(ml_dtypes.bfloat16))
        in_maps.append({
            "xqT": xqTc, "xkT": xkTc, "xvT": xvTc,
            "wqT": wqT, "wkT": wkT, "wvT": wvT, "woT": woT,
            "bq8": bq8, "bk_l": bk_l, "bo2": bo2,
            "notmT": notm,
        })

    nc = _get_nc()
    res = run_bass_kernel_spmd(nc, in_maps, core_ids=list(range(N_CORES)))
    out = np.empty((4, 2048, EMBED), dtype=np.float32)
    for c in range(N_CORES):
        b, h = c // 2, c % 2
        out[b, h * SQ:(h + 1) * SQ, :] = res.results[c]["outT"].T
    return out


# revision 42
# speedup vs baseline: 1.0097x; 1.0097x over previous
"""MultiHeadAttention Trainium2 Bass kernel, 8-core SPMD.

Problem: B=4, S=2048, EMBED=1024, HEADS=16, HEAD_DIM=64 (fp32).
Sharding: core c -> batch b=c//2, query-half h=c%2 (1024 query rows).
Each core computes its 1024 output rows end-to-end; no collectives.

Per-core dataflow (all layouts transposed: feature/kk dim on partitions):
  A1: KT = WkT.T @ XkT + bk              -> SBUF resident  (f32r)
  A2: V  = XvT.T @ WvT (natural [kk,d])  -> SBUF resident bf16, with a
      ones column per head (V_aug) so the PV matmul also produces the
      softmax denominator for free.
  B:  per head-pair p:
      Q-proj for the pair (QT tile stays in SBUF; scale 1/8 + bias
      folded into the PSUM evacuation), then per q-chunk qc(512) and
      kk-pair kkp:
        S.T[kk,q] = KT_h.T @ QT_h        (f32r, heads row-packed in PE)
        P = exp(S.T)                      (ACT, no max-subtraction --
                                           scores ~ N(0,1), safe)
        P *= (1-mask).T                   (DVE, bf16 2x mode)
        OT_h[65,512] += V_aug_h.T @ P     (bf16; row 64 = denominator)
      normalize: OT_h[0:64] * recip(OT_h[64]) -> ot_dram (f32r)
  C:  outT = WoT.T @ OT + (bo + Wo @ bv)  -> DRAM [1024,1024]
Host reassembles out[b, h*1024:(h+1)*1024, :] = outT.T per core.
"""
import numpy as np
import ml_dtypes

import concourse.bass as bass
import concourse.mybir as mybir
import concourse.tile as tile
from concourse import bacc
from concourse.bass_utils import run_bass_kernel_spmd

F32R = mybir.dt.float32r
F32 = mybir.dt.float32
BF16 = mybir.dt.bfloat16
Act = mybir.ActivationFunctionType
Alu = mybir.AluOpType

EMBED = 1024
HEADS = 16
DH = 64
SQ = 1024   # query rows per core
SK = 2048   # key rows per core
NF = 8      # feature tiles (1024/128)
NKT = 16    # kk tiles (2048/128)
N_CORES = 8

_STATE = {}


def build_nc():
    nc = bacc.Bacc("TRN2", target_bir_lowering=False)
    xqT = nc.dram_tensor("xqT", [EMBED, SQ], F32R, kind="ExternalInput")
    xkT = nc.dram_tensor("xkT", [EMBED, SK], F32R, kind="ExternalInput")
    xvT = nc.dram_tensor("xvT", [EMBED, SK], F32R, kind="ExternalInput")
    wqT = nc.dram_tensor("wqT", [EMBED, EMBED], F32R, kind="ExternalInput")
    wkT = nc.dram_tensor("wkT", [EMBED, EMBED], F32R, kind="ExternalInput")
    wvT = nc.dram_tensor("wvT", [EMBED, EMBED], F32R, kind="ExternalInput")
    woT = nc.dram_tensor("woT", [EMBED, EMBED], F32R, kind="ExternalInput")
    bq8 = nc.dram_tensor("bq8", [128, NF], F32, kind="ExternalInput")
    bk_l = nc.dram_tensor("bk_l", [128, NF], F32, kind="ExternalInput")
    bo2 = nc.dram_tensor("bo2", [128, NF], F32, kind="ExternalInput")
    notmT = nc.dram_tensor("notmT", [SK, SQ], BF16, kind="ExternalInput")
    outT = nc.dram_tensor("outT", [EMBED, SQ], F32, kind="ExternalOutput")
    ot_dram = nc.dram_tensor("ot_dram", [EMBED, SQ], F32R)

    xqT_r = xqT.rearrange("(t p) q -> p t q", p=128)
    xkT_r = xkT.rearrange("(t p) k -> p t k", p=128)
    xvT_r = xvT.rearrange("(t p) k -> p t k", p=128)
    wqT_r = wqT.rearrange("(t p) n -> p t n", p=128)
    wkT_r = wkT.rearrange("(t p) n -> p t n", p=128)
    wvT_r = wvT.rearrange("(t p) n -> p t n", p=128)
    woT_r = woT.rearrange("(t p) n -> p t n", p=128)
    notmT_r = notmT.rearrange("(t p) q -> p t q", p=128)
    ot_dram_r = ot_dram.rearrange("(t p) q -> p t q", p=128)

    with tile.TileContext(nc) as tc:
        # ---------- persistent + early-prefetch pools ----------
        with tc.tile_pool(name="persist", bufs=1) as pp, \
             tc.tile_pool(name="bias", bufs=1) as bp, \
             tc.tile_pool(name="xv", bufs=2) as xvpool, \
             tc.tile_pool(name="bwq", bufs=2) as wqpool:
            kt = pp.tile([128, NF, SK], F32R, name="kt")
            vaug = pp.tile([128, NKT, HEADS * 65], BF16, name="vaug")
            bq8_sb = bp.tile([128, NF], F32, name="bq8_sb")
            bk_sb = bp.tile([128, NF], F32, name="bk_sb")
            bo2_sb = bp.tile([128, NF], F32, name="bo2_sb")

            # ---------- phase A1: K projection -> kt ----------
            with tc.tile_pool(name="a1w", bufs=1) as wpool, \
                 tc.tile_pool(name="a1x", bufs=2) as xpool, \
                 tc.tile_pool(name="a1p", bufs=4, space="PSUM") as pspool:
                wk_sb = wpool.tile([128, NF, EMBED], F32R, name="wk_sb")
                for ck in range(4):
                    xk_sb = xpool.tile([128, NF, 512], F32R, name="xk_sb")
                    if ck == 0:
                        nc.sync.dma_start(out=xk_sb[:, 0:2, :],
                                          in_=xkT_r[:, 0:2, 0:512])
                        nc.sync.dma_start(out=wk_sb[:, :, 0:128],
                                          in_=wkT_r[:, :, 0:128])
                        nc.sync.dma_start(out=xk_sb[:, 2:4, :],
                                          in_=xkT_r[:, 2:4, 0:512])
                        nc.sync.dma_start(out=xk_sb[:, 4:8, :],
                                          in_=xkT_r[:, 4:8, 0:512])
                        nc.sync.dma_start(out=wk_sb[:, :, 128:256],
                                          in_=wkT_r[:, :, 128:256])
                        nc.sync.dma_start(out=bk_sb[:], in_=bk_l[:, :])
                        nc.sync.dma_start(out=bq8_sb[:], in_=bq8[:, :])
                        nc.sync.dma_start(out=bo2_sb[:], in_=bo2[:, :])
                        for c4 in range(1, 4):
                            nc.sync.dma_start(
                                out=wk_sb[:, :, c4 * 256:(c4 + 1) * 256],
                                in_=wkT_r[:, :, c4 * 256:(c4 + 1) * 256])
                    else:
                        nc.sync.dma_start(
                            out=xk_sb[:],
                            in_=xkT_r[:, :, ck * 512:(ck + 1) * 512])
                    for m in range(NF):
                        ps = pspool.tile([128, 512], F32, name="a1ps")
                        for fi in range(NF):
                            nc.tensor.matmul(
                                ps[:], wk_sb[:, fi, m * 128:(m + 1) * 128],
                                xk_sb[:, fi, :],
                                start=(fi == 0), stop=(fi == NF - 1))
                        nc.vector.tensor_scalar(
                            out=kt[:, m, ck * 512:(ck + 1) * 512],
                            in0=ps[:], scalar1=bk_sb[:, m:m + 1],
                            scalar2=None, op0=Alu.add)

            # ---------- phase A2: V projection -> vaug (bf16 + ones) ----------
            # n-outer: heads 0..7 (n=0) complete first so phase B's first
            # head-pairs can overlap with the n=1 half.
            vaug_r = vaug.rearrange("p k (h c) -> p k h c", c=65)
            nc.vector.memset(vaug_r[:, :, :, 64:65], 1.0)
            xq_ctx = tc.tile_pool(name="xq", bufs=1)
            xqpool = xq_ctx.__enter__()
            xq_sb = xqpool.tile([128, NF, SQ], F32R, name="xq_sb")
            nc.sync.dma_start(out=xq_sb[:, :, 0:512], in_=xqT_r[:, :, 0:512])
            nc.sync.dma_start(out=xq_sb[:, :, 512:1024],
                              in_=xqT_r[:, :, 512:1024])
            with tc.tile_pool(name="a2w", bufs=2) as wpool, \
                 tc.tile_pool(name="a2p", bufs=4, space="PSUM") as pspool:
                wv_sb = []
                for n in range(2):
                    t = wpool.tile([128, NF, 512], F32R, name="wv_sb")
                    nc.sync.dma_start(out=t[:],
                                      in_=wvT_r[:, :, n * 512:(n + 1) * 512])
                    wv_sb.append(t)
                for m in range(NKT):
                    xv_sb = xvpool.tile([128, NF, 128], F32R,
                                        name="xv_sb")
                    nc.sync.dma_start(
                        out=xv_sb[:],
                        in_=xvT_r[:, :, m * 128:(m + 1) * 128])
                    for n in range(2):
                        ps = pspool.tile([128, 512], F32, name="a2ps")
                        for fi in range(NF):
                            nc.tensor.matmul(
                                ps[:], xv_sb[:, fi, :],
                                wv_sb[n][:, fi, :],
                                start=(fi == 0), stop=(fi == NF - 1))
                        nc.vector.tensor_copy(
                            out=vaug_r[:, m, n * 8:(n + 1) * 8, 0:64],
                            in_=ps.rearrange("p (h c) -> p h c", c=64))

            # ---------- phase B: Q-proj + attention, per head pair ----------
            with tc.tile_pool(name="bnotm", bufs=1) as nmpool, \
                 tc.tile_pool(name="bqt", bufs=2) as qpool, \
                 tc.tile_pool(name="bpt", bufs=2) as ptpool, \
                 tc.tile_pool(name="bnrm", bufs=2) as npool, \
                 tc.tile_pool(name="bst", bufs=1, space="PSUM") as stpool, \
                 tc.tile_pool(name="bqp", bufs=2, space="PSUM") as qppool, \
                 tc.tile_pool(name="bot", bufs=1, space="PSUM") as otpool:
                wq_first = wqpool.tile([128, NF, 128], F32R, name="wq_sb",
                                       tag="wq_sb")
                nc.sync.dma_start(out=wq_first[:], in_=wqT_r[:, :, 0:128])
                notm = nmpool.tile([128, NKT, SQ], BF16, name="notm")
                for c4 in range(4):
                    nc.sync.dma_start(
                        out=notm[:, c4 * 4:(c4 + 1) * 4, :],
                        in_=notmT_r[:, c4 * 4:(c4 + 1) * 4, :])
                for p in range(8):  # head pairs
                    # Q projection for this pair -> qt_sb [128, 1024] f32r
                    if p == 0:
                        wq_sb = wq_first
                    else:
                        wq_sb = wqpool.tile([128, NF, 128], F32R,
                                            name="wq_sb", tag="wq_sb")
                        nc.sync.dma_start(
                            out=wq_sb[:],
                            in_=wqT_r[:, :, p * 128:(p + 1) * 128])
                    qt_sb = qpool.tile([128, SQ], F32R, name="qt_sb")
                    for qc in range(2):
                        qps = qppool.tile([128, 512], F32, name="qps")
                        for fi in range(NF):
                            nc.tensor.matmul(
                                qps[:], wq_sb[:, fi, :],
                                xq_sb[:, fi, qc * 512:(qc + 1) * 512],
                                start=(fi == 0), stop=(fi == NF - 1))
                        nc.vector.tensor_scalar(
                            out=qt_sb[:, qc * 512:(qc + 1) * 512], in0=qps[:],
                            scalar1=0.125, scalar2=bq8_sb[:, p:p + 1],
                            op0=Alu.mult, op1=Alu.add)
                    for qc in range(2):
                        otps = [otpool.tile([128, 512], F32,
                                            name=f"otps{j}", tag=f"otps{j}")
                                for j in range(2)]
                        for kkp in range(8):
                            sts = [stpool.tile([128, 1024], F32,
                                               name=f"stps{j}", tag=f"stps{j}")
                                   for j in range(2)]
                            # ST matmuls interleaved by head so adjacent
                            # PE ops target disjoint row groups (0,0)/(64,0)
                            # and run concurrently (MMs are strict FIFO --
                            # same-row-group neighbors serialize).
                            def st_mm(hh, j):
                                lo = hh * 64
                                kkt = 2 * kkp + j
                                nc.tensor.matmul(
                                    sts[hh][:, j * 512:(j + 1) * 512],
                                    kt[lo:lo + 64, p,
                                       kkt * 128:(kkt + 1) * 128],
                                    qt_sb[lo:lo + 64,
                                          qc * 512:(qc + 1) * 512],
                                    start=True, stop=True,
                                    tile_position=(lo, 0))
                            pts = []
                            st_mm(0, 0)
                            st_mm(1, 0)
                            st_mm(0, 1)
                            pt0 = ptpool.tile([128, 1024], BF16,
                                              name="pt0", tag="pt0")
                            nc.scalar.activation(pt0[:], sts[0][:], Act.Exp)
                            pts.append(pt0)
                            st_mm(1, 1)
                            pt1 = ptpool.tile([128, 1024], BF16,
                                              name="pt1", tag="pt1")
                            nc.scalar.activation(pt1[:], sts[1][:], Act.Exp)
                            pts.append(pt1)
                            for hh in range(2):
                                h = 2 * p + hh
                                for j in range(2):
                                    kkt = 2 * kkp + j
                                    nc.vector.tensor_tensor(
                                        out=pts[hh][:, j * 512:(j + 1) * 512],
                                        in0=pts[hh][:, j * 512:(j + 1) * 512],
                                        in1=notm[:, kkt,
                                                 qc * 512:(qc + 1) * 512],
                                        op=Alu.mult)
                                    nc.tensor.matmul(
                                        otps[hh][0:65, :],
                                        vaug_r[:, kkt, h, :],
                                        pts[hh][:, j * 512:(j + 1) * 512],
                                        start=(kkp == 0 and j == 0),
                                        stop=(kkp == 7 and j == 1))
                        for hh in range(2):
                            rec = npool.tile([1, 512], F32, name="rec",
                                             tag="rec")
                            nc.vector.reciprocal(rec[:], otps[hh][64:65, :])
                            recb = npool.tile([64, 512], F32, name="recb",
                                              tag="recb")
                            nc.gpsimd.partition_broadcast(recb[:], rec[:])
                            otstg = npool.tile([64, 512], F32R, name="otstg",
                                               tag="otstg")
                            nc.vector.tensor_tensor(
                                out=otstg[:],
                                in0=otps[hh][0:64, :], in1=recb[:],
                                op=Alu.mult)
                            nc.sync.dma_start(
                                out=ot_dram[p * 128 + hh * 64:
                                            p * 128 + hh * 64 + 64,
                                            qc * 512:(qc + 1) * 512],
                                in_=otstg[:])

            # ---------- phase C: output projection ----------
            # wo streams through the (still open) bwq pool so the first
            # blocks prefetch during phase B's tail.
            with tc.tile_pool(name="cot", bufs=1) as cotpool, \
                 tc.tile_pool(name="cs", bufs=3) as spool, \
                 tc.tile_pool(name="cp", bufs=4, space="PSUM") as pspool:
                ot_sb = []
                for qc in range(2):
                    t = cotpool.tile([128, NF, 512], F32R, name=f"ot_sb{qc}")
                    if qc == 0:
                        nc.sync.dma_start(out=t[:, 0:4, :],
                                          in_=ot_dram_r[:, 0:4, 0:512])
                        nc.sync.dma_start(out=t[:, 4:8, :],
                                          in_=ot_dram_r[:, 4:8, 0:512])
                    else:
                        nc.sync.dma_start(
                            out=t[:],
                            in_=ot_dram_r[:, :, qc * 512:(qc + 1) * 512])
                    ot_sb.append(t)
                for m in range(NF):
                    wo_sb = wqpool.tile([128, NF, 128], F32R, name="wo_sb",
                                        tag="wq_sb")
                    nc.sync.dma_start(
                        out=wo_sb[:],
                        in_=woT_r[:, :, m * 128:(m + 1) * 128])
                    for qc in range(2):
                        ps = pspool.tile([128, 512], F32, name="cps")
                        for fi in range(NF):
                            nc.tensor.matmul(
                                ps[:], wo_sb[:, fi, :],
                                ot_sb[qc][:, fi, :],
                                start=(fi == 0), stop=(fi == NF - 1))
                        stg = spool.tile([128, 512], F32, name="cstg")
                        nc.vector.tensor_scalar(
                            out=stg[:], in0=ps[:],
                            scalar1=bo2_sb[:, m:m + 1], scalar2=None,
                            op0=Alu.add)
                        nc.sync.dma_start(
                            out=outT[m * 128:(m + 1) * 128,
                                     qc * 512:(qc + 1) * 512],
                            in_=stg[:])
            xq_ctx.__exit__(None, None, None)
    nc.compile()
    return nc


def _get_nc():
    if "nc" not in _STATE:
        _STATE["nc"] = build_nc()
    return _STATE["nc"]


def kernel(query, key, value, mask, Wq, bq, Wk, bk, Wv, bv, Wo, bo):
    query = np.asarray(query, dtype=np.float32)
    key = np.asarray(key, dtype=np.float32)
    value = np.asarray(value, dtype=np.float32)
    mask = np.asarray(mask)
    Wq = np.asarray(Wq, dtype=np.float32)
    Wk = np.asarray(Wk, dtype=np.float32)
    Wv = np.asarray(Wv, dtype=np.float32)
    Wo = np.asarray(Wo, dtype=np.float32)
    bq = np.asarray(bq, dtype=np.float32)
    bk = np.asarray(bk, dtype=np.float32)
    bv = np.asarray(bv, dtype=np.float32)
    bo = np.asarray(bo, dtype=np.float32)

    wqT = np.ascontiguousarray(Wq.T)
    wkT = np.ascontiguousarray(Wk.T)
    wvT = np.ascontiguousarray(Wv.T)
    woT = np.ascontiguousarray(Wo.T)
    bq8 = np.ascontiguousarray((bq / 8.0).reshape(NF, 128).T)
    bk_l = np.ascontiguousarray(bk.reshape(NF, 128).T)
    bo2v = bo + Wo @ bv
    bo2 = np.ascontiguousarray(bo2v.reshape(NF, 128).T)

    in_maps = []
    for c in range(N_CORES):
        b, h = c // 2, c % 2
        rows = slice(h * SQ, (h + 1) * SQ)
        xqTc = np.ascontiguousarray(query[b, rows, :].T)
        xkTc = np.ascontiguousarray(key[b].T)
        xvTc = np.ascontiguousarray(value[b].T)
        notm = np.ascontiguousarray(
            (~mask[b, 0, rows, :]).T.astype(ml_dtypes.bfloat16))
        in_maps.append({
            "xqT": xqTc, "xkT": xkTc, "xvT": xvTc,
            "wqT": wqT, "wkT": wkT, "wvT": wvT, "woT": woT,
            "bq8": bq8, "bk_l": bk_l, "bo2": bo2,
            "notmT": notm,
        })

    nc = _get_nc()
    res = run_bass_kernel_spmd(nc, in_maps, core_ids=list(range(N_CORES)))
    out = np.empty((4, 2048, EMBED), dtype=np.float32)
    for c in range(N_CORES):
        b, h = c // 2, c % 2
        out[b, h * SQ:(h + 1) * SQ, :] = res.results[c]["outT"].T
    return out
